# revision 1
# baseline (speedup 1.0000x reference)
"""Trainium2 Bass kernel for the CNN-TRX few-shot attention head.

Sharding: data-parallel over the 200 queries (25 per NeuronCore); support set
and weights replicated per core. All matmuls in bf16 with fp32 PSUM:

  1. Frame projection in transposed layout: f_T[d, frame] for all 6 weight
     blocks (k_w/v_w x 3 tuple positions); biases folded via an augmented
     ones-row of X.
  2. Tuple gather (C(8,3)=56 frame triples) as 2-stage DVE column adds.
  3. LayerNorm of K projections column-wise: stats via ones-matmuls, Rsqrt on
     ACT, gpsimd partition-broadcast, two DVE passes.
  4. scoresT = s_k_pad^T q_k with supports sorted by class and class blocks
     padded to 128 rows; exp via ACT (no max-subtract: LN'd scores are O(1),
     Cauchy-Schwarz bounds |score| <= 34 so exp stays finite in fp32).
  5. Per-class prototypes in T-layout; distance terms ||q_v||^2, <q_v,P>,
     ||P||^2, sum(exp) via ones-matmul column reductions; final combine on
     single-partition rows; logits = -sum_a dist / 56.
"""

import math
from itertools import combinations

import ml_dtypes
import numpy as np

SEQ = 8
IN_DIM = 2048
OUT_DIM = 1152
TSS = 3
WAY = 5
N_SUPPORT = 25
N_QUERIES = 200
PE_SCALE = 0.1
LN_EPS = 1e-5
T = 56
N_CORES = 8
NQL = N_QUERIES // N_CORES      # queries per core
G_Q = 5                         # queries per inner group
N_GROUPS = NQL // G_Q
C = G_Q * T                     # score columns per group (280)
KPAD = 2176                     # 17 * 128 (2048 data + ones row + zero pad)
NKCH = KPAD // 128
NDCH = OUT_DIM // 128           # 9
NMB = 6 * OUT_DIM // 128        # 54 projection column blocks
NX = SEQ * 2 * N_SUPPORT        # 400 frame columns per core
PAIRS = [(t0, t1) for t0 in range(SEQ - 2) for t1 in range(t0 + 1, SEQ - 1)]
LN_CHUNK = 448                  # LayerNorm column chunk (PSUM free-dim <= 512)
BF16 = ml_dtypes.bfloat16

_CACHE = {}


def _pos_encoding():
    pos = np.arange(SEQ, dtype=np.float32)[:, None]
    div = np.exp(np.arange(0, IN_DIM, 2, dtype=np.float32) * -(math.log(10000.0) / IN_DIM))
    pe = np.zeros((SEQ, IN_DIM), dtype=np.float32)
    pe[:, 0::2] = np.sin(pos * div) * PE_SCALE
    pe[:, 1::2] = np.cos(pos * div) * PE_SCALE
    return pe


def _class_layout(counts):
    offs, off = [], 0
    for c in range(WAY):
        offs.append(off)
        off += ((counts[c] * T + 127) // 128) * 128
    return offs, off


def _build_kernel(counts, trivial_gb):
    import concourse.mybir as mybir
    import concourse.tile as tile
    from concourse import bacc
    from concourse.masks import make_identity

    f32 = mybir.dt.float32
    bf16 = mybir.dt.bfloat16
    AF = mybir.ActivationFunctionType
    ALU = mybir.AluOpType
    offs, nb_pad = _class_layout(counts)
    nwch = nb_pad // 128
    inv_sqrt = 1.0 / math.sqrt(OUT_DIM)

    nc = bacc.Bacc("TRN2", target_bir_lowering=False, debug=False,
                   enable_asserts=False, num_devices=N_CORES)

    x_d = nc.dram_tensor("x", [128, NKCH, NX], bf16, kind="ExternalInput").ap()
    w_d = nc.dram_tensor("w", [128, NMB, NKCH, 128], bf16, kind="ExternalInput").ap()
    g_d = nc.dram_tensor("lng", [128, NDCH], bf16, kind="ExternalInput").ap()
    b_d = nc.dram_tensor("lnb", [128, NDCH], bf16, kind="ExternalInput").ap()
    out_d = nc.dram_tensor("out", [NQL, WAY], f32, kind="ExternalOutput").ap()

    with tile.TileContext(nc) as tc:
        with tc.tile_pool(name="big", bufs=1) as big, \
             tc.tile_pool(name="small", bufs=1) as small:
            # frame projections, T-layout; one tile per weight block so
            # consumers only wait for the blocks they read (Tile tracks
            # dependencies per tile, not per region)
            f_b = [big.tile([128, NDCH, NX], bf16, name=f"f_b{j}") for j in range(6)]
            s_kT = big.tile([128, NDCH, nb_pad], bf16)      # LN'd support K, padded cols
            s_v = big.tile([128, nwch, OUT_DIM], bf16)      # support V, row-natural padded
            ones_sb = small.tile([128, 1], bf16)
            nc.vector.memset(ones_sb, 1.0)
            eps_sb = small.tile([1, 1], f32)
            nc.vector.memset(eps_sb, LN_EPS)
            g_sb = small.tile([128, NDCH], bf16)
            b_sb = small.tile([128, NDCH], bf16)
            nc.sync.dma_start(g_sb, g_d)
            nc.sync.dma_start(b_sb, b_d)
            logits5 = small.tile([WAY, NQL], f32)

            # ---------- Phase 1: frame projections ----------
            # sprep/pp_t open BEFORE the phase-1 pools so their SBUF/PSUM space
            # does not alias xt/xw: otherwise the support-side gathers inherit
            # address-level WAW deps on the last projection and cannot overlap.
            sprep_cm = tc.tile_pool(name="sprep", bufs=2)
            pp_t_cm = tc.tile_pool(name="pp_t", bufs=4, space="PSUM")
            sprep = sprep_cm.__enter__()
            pp_t = pp_t_cm.__enter__()
            with tc.tile_pool(name="xt_pool", bufs=1) as xt_pool, \
                 tc.tile_pool(name="xw", bufs=3) as xw, \
                 tc.tile_pool(name="pp_proj", bufs=4, space="PSUM") as pp_proj:
                xt = xt_pool.tile([128, NKCH, NX], bf16)
                nc.sync.dma_start(xt, x_d)
                for m in range(NMB):
                    wm = xw.tile([128, NKCH, 128], bf16, tag="wslab")
                    nc.sync.dma_start(wm, w_d[:, m])
                    ps = pp_proj.tile([128, NX], f32, tag="projps")
                    for k in range(NKCH):
                        nc.tensor.matmul(ps, wm[:, k], xt[:, k],
                                         start=(k == 0), stop=(k == NKCH - 1))
                    nc.scalar.activation(f_b[m // NDCH][:, m % NDCH], ps, AF.Copy)

            f_i = [fb.rearrange("p d (i s) -> p d i s", s=SEQ) for fb in f_b]

            def gather_one(dst4, kv, items0, n_items, pool):
                """dst4 [128, NDCH, n_items, T] = tuple-gathered frame
                projections for one path (kv=0: K blocks 0-2, kv=1: V 3-5)."""
                isl = slice(items0, items0 + n_items)
                b0, b1, b2 = (f_i[3 * kv + j] for j in range(TSS))
                p2 = pool.tile([128, NDCH, n_items, len(PAIRS)], bf16,
                               tag=f"pairs{kv}", name="p2")
                pi = 0
                for t0 in range(SEQ - 2):
                    run = SEQ - 2 - t0
                    a = b0[:, :, isl, t0:t0 + 1]
                    b = b1[:, :, isl, t0 + 1:t0 + 1 + run]
                    nc.vector.tensor_add(p2[:, :, :, pi:pi + run],
                                         a.to_broadcast(b.shape), b)
                    pi += run
                ai = 0
                for pi, (t0, t1) in enumerate(PAIRS):
                    run = SEQ - 1 - t1
                    a = p2[:, :, :, pi:pi + 1]
                    b = b2[:, :, isl, t1 + 1:t1 + 1 + run]
                    nc.vector.tensor_add(dst4[:, :, :, ai:ai + run],
                                         a.to_broadcast(b.shape), b)
                    ai += run

            def col_ln(raw, cols, pool, psum_pool, out=None):
                """Column-wise LayerNorm of raw [128, NDCH, cols] (T-layout);
                in place unless `out` is given."""
                if out is None:
                    out = raw
                for c0 in range(0, cols, LN_CHUNK):
                    cw = min(LN_CHUNK, cols - c0)
                    r = raw[:, :, c0:c0 + cw]
                    o = out[:, :, c0:c0 + cw]
                    sq = pool.tile([128, NDCH, cw], bf16, tag="lnsq", name="lnsq", bufs=1)
                    nc.scalar.activation(sq, r, AF.Square)
                    ps_s = psum_pool.tile([1, cw], f32, tag="lnps", name="lnps")
                    ps_q = psum_pool.tile([1, cw], f32, tag="lnps", name="lnps")
                    for k in range(NDCH):
                        nc.tensor.matmul(ps_s, ones_sb, r[:, k],
                                         start=(k == 0), stop=(k == NDCH - 1))
                    for k in range(NDCH):
                        nc.tensor.matmul(ps_q, ones_sb, sq[:, k],
                                         start=(k == 0), stop=(k == NDCH - 1))
                    m_r = pool.tile([1, cw], f32, tag="lnm", name="lnm")
                    v_r = pool.tile([1, cw], f32, tag="lnv", name="lnv")
                    mm = pool.tile([1, cw], f32, tag="lnmm", name="lnmm")
                    nc.scalar.activation(m_r, ps_s, AF.Copy, scale=1.0 / OUT_DIM)
                    nc.scalar.activation(v_r, ps_q, AF.Copy, scale=1.0 / OUT_DIM)
                    nc.vector.tensor_mul(mm, m_r, m_r)
                    nc.vector.tensor_sub(v_r, v_r, mm)
                    nc.scalar.activation(v_r, v_r, AF.Sqrt, bias=eps_sb)
                    nc.vector.reciprocal(v_r, v_r)
                    # bf16 broadcast operands keep the big apply passes in the
                    # DVE 16-bit fast path
                    m_h = pool.tile([1, cw], bf16, tag="lnmh", name="lnmh")
                    v_h = pool.tile([1, cw], bf16, tag="lnvh", name="lnvh")
                    nc.vector.tensor_copy(m_h, m_r)
                    nc.vector.tensor_copy(v_h, v_r)
                    m_b = pool.tile([128, cw], bf16, tag="lnmb", name="lnmb", bufs=1)
                    a_b = pool.tile([128, cw], bf16, tag="lnab", name="lnab", bufs=1)
                    nc.gpsimd.partition_broadcast(m_b, m_h)
                    nc.gpsimd.partition_broadcast(a_b, v_h)
                    mb3 = m_b[:, None, :].to_broadcast([128, NDCH, cw])
                    ab3 = a_b[:, None, :].to_broadcast([128, NDCH, cw])
                    nc.vector.tensor_sub(r, r, mb3)
                    nc.vector.tensor_mul(o, r, ab3)
                    if not trivial_gb:
                        for k in range(NDCH):
                            nc.vector.tensor_scalar(o[:, k], o[:, k],
                                                    g_sb[:, k:k + 1], b_sb[:, k:k + 1],
                                                    ALU.mult, ALU.add)

            # ---------- Phase 2: support-side tensors ----------
            # sprep coexists with the phase-1 pools (opened in the same scope,
            # before phase-1 pools closed above would reuse its space) so the
            # gathers run on the idle DVE while projections stream on the PE.
            # K path gathers straight into the persistent s_kT (LN in place);
            # V path gathers into a small per-class scratch, transposed
            # class-by-class into s_v.
            if True:
                ident = small.tile([128, 128], bf16)
                make_identity(nc, ident)
                max_ch = max((int(counts[c]) * T + 127) // 128 for c in range(WAY))
                with tc.tile_pool(name="pp_s", bufs=2, space="PSUM") as pp_s:
                    # K path first (gather -> per-class LN) so scores can start
                    # as soon as possible; V gathers/transposes follow and
                    # overlap the first group's score matmuls.
                    start_item = 0
                    for c in range(WAY):
                        n_c = int(counts[c])
                        rows = n_c * T
                        pad_lo = offs[c] + rows
                        pad_hi = offs[c + 1] if c + 1 < WAY else nb_pad
                        if pad_hi > pad_lo:
                            nc.gpsimd.memset(s_kT[:, :, pad_lo:pad_hi], 0.0)
                        dst_k = s_kT[:, :, offs[c]:offs[c] + rows].rearrange(
                            "p m (n a) -> p m n a", a=T)
                        gather_one(dst_k, 0, start_item, n_c, sprep)
                        col_ln(s_kT[:, :, offs[c]:offs[c] + rows], rows, sprep, pp_s)
                        start_item += n_c
                    start_item = 0
                    for c in range(WAY):
                        n_c = int(counts[c])
                        rows = n_c * T
                        nch = (rows + 127) // 128
                        wlo = offs[c] // 128
                        s_vT_c = sprep.tile([128, NDCH, max_ch * 128], bf16,
                                            tag="svtc", name="svtc")
                        if rows < nch * 128:
                            nc.gpsimd.memset(s_vT_c[:, :, rows:nch * 128], 0.0)
                        dst_v = s_vT_c[:, :, :rows].rearrange("p m (n a) -> p m n a", a=T)
                        gather_one(dst_v, 1, start_item, n_c, sprep)
                        for w in range(nch):
                            for dd in range(NDCH):
                                ps = pp_t.tile([128, 128], bf16, tag="tps")
                                nc.tensor.transpose(
                                    ps, s_vT_c[:, dd, w * 128:(w + 1) * 128], ident)
                                if dd % 2 == 0:
                                    nc.vector.tensor_copy(
                                        s_v[:, wlo + w, dd * 128:(dd + 1) * 128], ps)
                                else:
                                    nc.scalar.activation(
                                        s_v[:, wlo + w, dd * 128:(dd + 1) * 128],
                                        ps, AF.Copy)
                        start_item += n_c
            pp_t_cm.__exit__(None, None, None)
            sprep_cm.__exit__(None, None, None)

            # ---------- Phase 3: per-group query pipeline ----------
            # Column sums (S_c, B_c, C_c, A) are M=1 ones-matmuls packed 4 per
            # PSUM tile at partitions {0,32,64,96} via tile_position so the PE
            # runs them concurrently in distinct 32-column groups. The rows are
            # then DMA-packed into a [WAY, 4, C] tile (partition = class) so
            # the final combine runs on 5 lanes instead of 1.
            TS, TB, TC, TA = 0, 1, 2, 3  # term slots in the packed tile

            def packed_sum(ps_tile, slot, rhs_chunks, first, last):
                """Accumulate sum-over-partitions of each rhs chunk into
                ps_tile[32*slot] using a col-group tile_position."""
                out = ps_tile[32 * slot:32 * slot + 1]
                for i, (rhs, kc) in enumerate(rhs_chunks):
                    nc.tensor.matmul(out, ones_sb[:kc], rhs, start=(first and i == 0),
                                     stop=(last and i == len(rhs_chunks) - 1),
                                     tile_position=(0, 32 * slot),
                                     skip_group_check=True)

            with tc.tile_pool(name="grp", bufs=2) as grp, \
                 tc.tile_pool(name="rows", bufs=2) as rows_pool, \
                 tc.tile_pool(name="pp_sc", bufs=2, space="PSUM") as pp_sc, \
                 tc.tile_pool(name="pp_pr", bufs=2, space="PSUM") as pp_pr, \
                 tc.tile_pool(name="pp_row", bufs=2, space="PSUM") as pp_row:
                for g in range(N_GROUPS):
                    q_kT = grp.tile([128, NDCH, G_Q, T], bf16, tag="qk")
                    q_vT = grp.tile([128, NDCH, G_Q, T], bf16, tag="qv")
                    items0 = N_SUPPORT + g * G_Q
                    gather_one(q_kT, 0, items0, G_Q, grp)
                    gather_one(q_vT, 1, items0, G_Q, grp)
                    qk3 = q_kT.rearrange("p m q a -> p m (q a)")
                    qv3 = q_vT.rearrange("p m q a -> p m (q a)")
                    col_ln(qk3, C, grp, pp_row)

                    # scoresT + exp
                    exp_t = grp.tile([128, nwch, C], bf16, tag="exp")
                    for w in range(nwch):
                        ps = pp_sc.tile([128, C], f32, tag="scps")
                        for k in range(NDCH):
                            nc.tensor.matmul(ps, s_kT[:, k, w * 128:(w + 1) * 128],
                                             qk3[:, k], start=(k == 0), stop=(k == NDCH - 1))
                        nc.scalar.activation(exp_t[:, w], ps, AF.Exp, scale=inv_sqrt)

                    packed = rows_pool.tile([WAY, 4, C], f32, tag="packed")

                    # A = ||q_v||^2 per column
                    qsq = grp.tile([128, NDCH, C], bf16, tag="qsq", bufs=1)
                    nc.scalar.activation(qsq, qv3, AF.Square)
                    ps_a = pp_row.tile([128, C], f32, tag="sumps", name="ps_a")
                    packed_sum(ps_a, 0, [(qsq[:, k], 128) for k in range(NDCH)], True, True)
                    a_sb = rows_pool.tile([1, C], f32, tag="a_sb", bufs=1)
                    nc.vector.tensor_copy(a_sb, ps_a[0:1])
                    a5 = rows_pool.tile([WAY, C], f32, tag="a5", bufs=1)
                    nc.gpsimd.partition_broadcast(a5, a_sb)

                    def s_chunks(c):
                        rows = int(counts[c]) * T
                        wlo = offs[c] // 128
                        return [(exp_t[:min(128, rows - wi * 128), wlo + wi],
                                 min(128, rows - wi * 128))
                                for wi in range((rows + 127) // 128)]

                    def stage_rows(ps, rows):
                        """PSUM sum-tile -> SBUF (one lane-parallel copy; engines
                        cannot address partition starts other than 0/32/64/96),
                        then DMA rows {32j} into packed[class, term]."""
                        st = rows_pool.tile([128, C], f32, tag="stage", name="stage")
                        nc.scalar.activation(st, ps, AF.Copy)
                        st4 = st.rearrange("(j z) n -> j z n", z=32)
                        for j, (cc, term) in enumerate(rows):
                            nc.sync.dma_start(packed[cc:cc + 1, term],
                                              st4[j:j + 1, 0, :])

                    # S_0..S_3 packed in one PSUM tile (concurrent col-groups)
                    ps_s03 = pp_row.tile([128, C], f32, tag="sumps", name="ps_s03")
                    nchunks = max(len(s_chunks(c)) for c in range(4))
                    for i in range(nchunks):
                        for c in range(4):
                            ch = s_chunks(c)
                            if i < len(ch):
                                packed_sum(ps_s03, c, [ch[i]], i == 0, i == len(ch) - 1)
                    stage_rows(ps_s03, [(0, TS), (1, TS), (2, TS), (3, TS)])

                    # remaining sum streams: S_4, then B_c/C_c per class,
                    # packed 4 per PSUM tile
                    ps_bc = pp_row.tile([128, C], f32, tag="sumps", name="ps_bc0")
                    packed_sum(ps_bc, 0, s_chunks(4), True, True)
                    pending = [(4, TS)]

                    def bc_flush(force=False):
                        nonlocal ps_bc, pending
                        if pending and (force or len(pending) >= 3):
                            stage_rows(ps_bc, pending)
                            pending = []
                            if not force:
                                ps_bc = pp_row.tile([128, C], f32, tag="sumps",
                                                    name="ps_bc")
                    for c in range(WAY):
                        rows = int(counts[c]) * T
                        wlo = offs[c] // 128
                        nw_c = (rows + 127) // 128
                        # prototypes: P[d, col] accumulated over class rows
                        pt = grp.tile([128, NDCH, C], bf16, tag="pt", bufs=1)
                        p2t = grp.tile([128, NDCH, C], bf16, tag="p2t", bufs=1)
                        for dd in range(NDCH):
                            ps_p = pp_pr.tile([128, C], f32, tag="prps")
                            for wi in range(nw_c):
                                nc.tensor.matmul(ps_p, s_v[:, wlo + wi, dd * 128:(dd + 1) * 128],
                                                 exp_t[:, wlo + wi],
                                                 start=(wi == 0), stop=(wi == nw_c - 1))
                            if dd % 2 == 0:
                                nc.scalar.activation(pt[:, dd], ps_p, AF.Copy)
                            else:
                                nc.vector.tensor_copy(pt[:, dd], ps_p)
                        nc.scalar.activation(p2t, pt, AF.Square)
                        nc.vector.tensor_mul(pt, pt, qv3)  # now <q_v, P> terms
                        # interleave B_c / C_c chunk streams for PE concurrency
                        sb, sc = len(pending), len(pending) + 1
                        for k in range(NDCH):
                            packed_sum(ps_bc, sb, [(pt[:, k], 128)], k == 0, k == NDCH - 1)
                            packed_sum(ps_bc, sc, [(p2t[:, k], 128)], k == 0, k == NDCH - 1)
                        pending += [(c, TB), (c, TC)]
                        bc_flush()
                    bc_flush(force=True)

                    # dist = A - 2 B/S + C/S^2 ; logits = -sum_a dist / T
                    sinv = rows_pool.tile([WAY, C], f32, tag="sinv", bufs=1)
                    nc.vector.reciprocal(sinv, packed[:, TS])
                    u = rows_pool.tile([WAY, C], f32, tag="u", bufs=1)
                    nc.vector.tensor_mul(u, packed[:, TC, :], sinv)
                    nc.vector.scalar_tensor_tensor(u, packed[:, TB, :], -2.0, u,
                                                   ALU.mult, ALU.add)
                    nc.vector.tensor_mul(u, u, sinv)
                    nc.vector.tensor_add(u, u, a5)
                    u4 = u.rearrange("w (q a) -> w q a", a=T)
                    red = rows_pool.tile([WAY, G_Q], f32, tag="red", bufs=1)
                    nc.vector.reduce_sum(red, u4, mybir.AxisListType.X)
                    nc.scalar.activation(logits5[:, g * G_Q:(g + 1) * G_Q], red,
                                         AF.Copy, scale=-1.0 / T)

            nc.sync.dma_start(out_d.rearrange("q c -> c q"), logits5)

    nc.compile()
    return nc


def kernel(support_set, support_labels, queries, k_w, k_b, v_w, v_b, ln_g, ln_b):
    import concourse.bass_utils as bass_utils

    support_set = np.asarray(support_set, dtype=np.float32)
    queries = np.asarray(queries, dtype=np.float32)
    labels = np.asarray(support_labels, dtype=np.int32)
    k_w = np.asarray(k_w, dtype=np.float32)
    v_w = np.asarray(v_w, dtype=np.float32)
    k_b = np.asarray(k_b, dtype=np.float32)
    v_b = np.asarray(v_b, dtype=np.float32)
    ln_g = np.asarray(ln_g, dtype=np.float32)
    ln_b = np.asarray(ln_b, dtype=np.float32)

    pe = _pos_encoding()
    s = support_set + pe[None]
    q = queries + pe[None]
    order = np.argsort(labels, kind="stable")
    counts = np.bincount(labels, minlength=WAY)
    s_sorted = s[order]
    trivial_gb = bool(np.all(ln_g == 1.0) and np.all(ln_b == 0.0))

    key = (tuple(int(x) for x in counts), trivial_gb)
    if key not in _CACHE:
        _CACHE[key] = _build_kernel(counts, trivial_gb)
    nc = _CACHE[key]

    W = np.zeros((KPAD, 6 * OUT_DIM), np.float32)
    for j in range(TSS):
        W[:IN_DIM, j * OUT_DIM:(j + 1) * OUT_DIM] = k_w[j * IN_DIM:(j + 1) * IN_DIM]
        W[:IN_DIM, (TSS + j) * OUT_DIM:(TSS + j + 1) * OUT_DIM] = v_w[j * IN_DIM:(j + 1) * IN_DIM]
        W[IN_DIM, j * OUT_DIM:(j + 1) * OUT_DIM] = k_b / TSS
        W[IN_DIM, (TSS + j) * OUT_DIM:(TSS + j + 1) * OUT_DIM] = v_b / TSS
    w_perm = np.ascontiguousarray(
        W.reshape(NKCH, 128, NMB, 128).transpose(1, 2, 0, 3)).astype(BF16)
    g_in = np.ascontiguousarray(ln_g.reshape(NDCH, 128).T).astype(BF16)
    b_in = np.ascontiguousarray(ln_b.reshape(NDCH, 128).T).astype(BF16)

    in_maps = []
    for core in range(N_CORES):
        qs = q[core * NQL:(core + 1) * NQL]
        X = np.concatenate([s_sorted.reshape(-1, IN_DIM), qs.reshape(-1, IN_DIM)], 0)
        XT = np.zeros((KPAD, NX), np.float32)
        XT[:IN_DIM] = X.T
        XT[IN_DIM] = 1.0
        x_perm = np.ascontiguousarray(
            XT.reshape(NKCH, 128, NX).transpose(1, 0, 2)).astype(BF16)
        in_maps.append({"x": x_perm, "w": w_perm, "lng": g_in, "lnb": b_in})

    global _LAST_IN_MAPS
    _LAST_IN_MAPS = in_maps
    res = bass_utils.run_bass_kernel_spmd(nc, in_maps, core_ids=list(range(N_CORES)))
    return np.concatenate([res.results[i]["out"] for i in range(N_CORES)], 0)


_LAST_IN_MAPS = None



# revision 9
# speedup vs baseline: 1.1863x; 1.1863x over previous
"""Trainium2 Bass kernel for the CNN-TRX few-shot attention head.

Sharding: data-parallel over the 200 queries (25 per NeuronCore); support set
and weights replicated per core. v2: fp8(e4m3) DoubleRow matmuls + merged
K/V gathers + 3 query groups.

  1. Frame projection in transposed layout, fp8 DoubleRow over 8 k-chunk
     pairs (contraction 2048 = 16 chunks); weights pre-scaled by 1024 on
     host; biases applied via the PSUM->SBUF activation (per-partition
     bias AP), V blocks additionally scaled by S_V=4 for later fp8 use.
  2. Tuple gather (C(8,3)=56 frame triples) as 2-stage DVE column adds,
     K and V paths merged into one 18-chunk-wide pass.
  3. Column LayerNorm: stats via 2-slot packed ones-matmuls, apply on DVE
     writing fp8 (scale 4) directly.
  4. scoresT = s_k^T q_k in fp8: 4 DoubleRow d-chunk pairs + 1 plain fp8
     matmul; exp via ACT with -ln(8) bias so fp8 exp stays in range.
  5. Per-class prototypes in T-layout, fp8 DoubleRow over class row-chunk
     pairs (+1 plain matmul for odd chunks); distance terms via packed
     ones-matmul column sums; final combine on 5-partition rows.
"""

import math
from itertools import combinations

import ml_dtypes
import numpy as np

SEQ = 8
IN_DIM = 2048
OUT_DIM = 1152
TSS = 3
WAY = 5
N_SUPPORT = 25
N_QUERIES = 200
PE_SCALE = 0.1
LN_EPS = 1e-5
T = 56
N_CORES = 8
NQL = N_QUERIES // N_CORES      # queries per core (25)
G_SIZES = [8, 8, 9]             # query group sizes (sum = NQL)
G_MAX = max(G_SIZES)
C_ALLOC = 512                   # column allocation per group (>= 9*56)
NKCH = IN_DIM // 128            # 16 contraction chunks
NDCH = OUT_DIM // 128           # 9
NMB = 6 * NDCH                  # 54 projection blocks (j, kv, dd)
NX = SEQ * 2 * N_SUPPORT        # 400 frame columns per core
PAIRS = [(t0, t1) for t0 in range(SEQ - 2) for t1 in range(t0 + 1, SEQ - 1)]
LN_CHUNK = 512
S_W = 1024.0                    # weight fp8 scale
S_K = 4.0                       # LN'd K fp8 scale
S_V = 4.0                       # V fp8 scale
EXP_SHIFT = math.log(8.0)       # exp output scale 1/8 (fp8 range)
BF16 = ml_dtypes.bfloat16
F8 = ml_dtypes.float8_e4m3

_CACHE = {}


def _pos_encoding():
    pos = np.arange(SEQ, dtype=np.float32)[:, None]
    div = np.exp(np.arange(0, IN_DIM, 2, dtype=np.float32) * -(math.log(10000.0) / IN_DIM))
    pe = np.zeros((SEQ, IN_DIM), dtype=np.float32)
    pe[:, 0::2] = np.sin(pos * div) * PE_SCALE
    pe[:, 1::2] = np.cos(pos * div) * PE_SCALE
    return pe


def _class_layout(counts):
    offs, off = [], 0
    for c in range(WAY):
        offs.append(off)
        off += ((counts[c] * T + 127) // 128) * 128
    return offs, off


def _build_kernel(counts, trivial_gb):
    import concourse.mybir as mybir
    import concourse.tile as tile
    from concourse import bacc
    from concourse.masks import make_identity

    f32 = mybir.dt.float32
    bf16 = mybir.dt.bfloat16
    fp8 = mybir.dt.float8e4
    AF = mybir.ActivationFunctionType
    ALU = mybir.AluOpType
    DR = mybir.MatmulPerfMode.DoubleRow
    offs, nb_pad = _class_layout(counts)
    nwch = nb_pad // 128
    inv_sqrt = 1.0 / math.sqrt(OUT_DIM)

    nc = bacc.Bacc("TRN2", target_bir_lowering=False, debug=False,
                   enable_asserts=False, num_devices=N_CORES)

    x_d = nc.dram_tensor("x", [128, NKCH, NX], fp8, kind="ExternalInput").ap()
    w_d = nc.dram_tensor("w", [128, NMB, NKCH, 128], fp8, kind="ExternalInput").ap()
    bias_d = nc.dram_tensor("bias", [128, NMB], f32, kind="ExternalInput").ap()
    g_d = nc.dram_tensor("lng", [128, NDCH], bf16, kind="ExternalInput").ap()
    b_d = nc.dram_tensor("lnb", [128, NDCH], bf16, kind="ExternalInput").ap()
    out_d = nc.dram_tensor("out", [NQL, WAY], f32, kind="ExternalOutput").ap()

    with tile.TileContext(nc) as tc:
        with tc.tile_pool(name="big", bufs=1) as big, \
             tc.tile_pool(name="small", bufs=1) as small:
            # frame projections, T-layout; one tile per tuple position j so
            # gather stage1 (j=0,1) can start before j=2 lands
            f_all = [big.tile([128, 18, NX], bf16, name=f"f_j{j}") for j in range(3)]
            s_kT = big.tile([128, NDCH, nb_pad], fp8)       # LN'd support K * S_K
            s_v = big.tile([128, nwch, OUT_DIM], fp8)       # support V * S_V, row-natural
            ones_bf = small.tile([128, 1], bf16)
            nc.vector.memset(ones_bf, 1.0)
            ones_f8 = small.tile([128, 1], fp8)
            nc.vector.memset(ones_f8, 1.0)
            eps_sb = small.tile([1, 1], f32)
            nc.vector.memset(eps_sb, LN_EPS)
            expb_sb = small.tile([128, 1], f32)
            nc.vector.memset(expb_sb, -EXP_SHIFT)
            g_sb = small.tile([128, NDCH], bf16)
            b_sb = small.tile([128, NDCH], bf16)
            bias_sb = small.tile([128, NMB], f32)
            nc.sync.dma_start(g_sb, g_d)
            nc.sync.dma_start(b_sb, b_d)
            nc.sync.dma_start(bias_sb, bias_d)
            logits5 = small.tile([WAY, NQL], f32)

            # ---------- Phase 1: frame projections (fp8 DoubleRow) ----------
            sprep_cm = tc.tile_pool(name="sprep", bufs=2)
            pp_t_cm = tc.tile_pool(name="pp_t", bufs=4, space="PSUM")
            sprep = sprep_cm.__enter__()
            pp_t = pp_t_cm.__enter__()
            with tc.tile_pool(name="xt_pool", bufs=1) as xt_pool, \
                 tc.tile_pool(name="xw", bufs=3) as xw, \
                 tc.tile_pool(name="pp_proj", bufs=4, space="PSUM") as pp_proj:
                xt = xt_pool.tile([128, NKCH, NX], fp8)
                nc.sync.dma_start(xt, x_d)
                for j in range(3):
                    for b in range(18):          # b = kv*9 + dd
                        m = j * 18 + b
                        wm = xw.tile([128, NKCH, 128], fp8, tag="wslab")
                        nc.sync.dma_start(wm, w_d[:, m])
                        ps = pp_proj.tile([128, NX], f32, tag="projps")
                        for k in range(NKCH // 2):
                            nc.tensor.matmul(ps, wm[:, 2 * k:2 * k + 2],
                                             xt[:, 2 * k:2 * k + 2],
                                             start=(k == 0), stop=(k == NKCH // 2 - 1),
                                             perf_mode=DR)
                        scale = (1.0 / S_W) if b < 9 else (S_V / S_W)
                        nc.scalar.activation(f_all[j][:, b], ps, AF.Identity,
                                             bias=bias_sb[:, m:m + 1], scale=scale)

            f_i = [fa.rearrange("p b (i s) -> p b i s", s=SEQ) for fa in f_all]

            def gather_kv(dst4, items0, n_items, pool):
                """dst4 [128, 18, n_items, T] = tuple-gathered K(0:9)+V(9:18)
                frame projections in one merged pass."""
                isl = slice(items0, items0 + n_items)
                b0, b1, b2 = f_i
                p2 = pool.tile([128, 18, n_items, len(PAIRS)], bf16,
                               tag="pairs", name="p2", bufs=1)
                pi = 0
                for t0 in range(SEQ - 2):
                    run = SEQ - 2 - t0
                    a = b0[:, :, isl, t0:t0 + 1]
                    b = b1[:, :, isl, t0 + 1:t0 + 1 + run]
                    nc.vector.tensor_add(p2[:, :, :, pi:pi + run],
                                         a.to_broadcast(b.shape), b)
                    pi += run
                ai = 0
                for pi, (t0, t1) in enumerate(PAIRS):
                    run = SEQ - 1 - t1
                    a = p2[:, :, :, pi:pi + 1]
                    b = b2[:, :, isl, t1 + 1:t1 + 1 + run]
                    nc.vector.tensor_add(dst4[:, :, :, ai:ai + run],
                                         a.to_broadcast(b.shape), b)
                    ai += run

            def col_ln(raw, cols, out_q, pool, psum_pool):
                """Column-wise LayerNorm of raw [128, NDCH, cols] bf16
                (T-layout, modified in place); writes fp8 out_q = LN(x)*S_K
                (gamma/beta folded in when nontrivial)."""
                for c0 in range(0, cols, LN_CHUNK):
                    cw = min(LN_CHUNK, cols - c0)
                    r = raw[:, :, c0:c0 + cw]
                    o = out_q[:, :, c0:c0 + cw]
                    sq = pool.tile([128, NDCH, cw], bf16, tag="lnsq", name="lnsq", bufs=1)
                    nc.vector.tensor_mul(sq, r, r)
                    ps2 = psum_pool.tile([64, cw], f32, tag="lnps", name="lnps")
                    for k in range(NDCH):
                        nc.tensor.matmul(ps2[0:1], ones_bf, r[:, k],
                                         start=(k == 0), stop=(k == NDCH - 1),
                                         tile_position=(0, 0), skip_group_check=True)
                        nc.tensor.matmul(ps2[32:33], ones_bf, sq[:, k],
                                         start=(k == 0), stop=(k == NDCH - 1),
                                         tile_position=(0, 32), skip_group_check=True)
                    m_r = pool.tile([1, cw], f32, tag="lnm", name="lnm")
                    v_r = pool.tile([1, cw], f32, tag="lnv", name="lnv")
                    mm = pool.tile([1, cw], f32, tag="lnmm", name="lnmm")
                    nc.scalar.activation(m_r, ps2[0:1], AF.Copy, scale=1.0 / OUT_DIM)
                    nc.scalar.activation(v_r, ps2[32:33], AF.Copy, scale=1.0 / OUT_DIM)
                    nc.vector.tensor_mul(mm, m_r, m_r)
                    nc.vector.tensor_sub(v_r, v_r, mm)
                    nc.scalar.activation(v_r, v_r, AF.Sqrt, bias=eps_sb)
                    nc.vector.reciprocal(v_r, v_r)
                    m_h = pool.tile([1, cw], bf16, tag="lnmh", name="lnmh")
                    v_h = pool.tile([1, cw], bf16, tag="lnvh", name="lnvh")
                    nc.vector.tensor_copy(m_h, m_r)
                    if trivial_gb:
                        nc.vector.tensor_scalar(v_h, v_r, S_K, None, ALU.mult)
                    else:
                        nc.vector.tensor_copy(v_h, v_r)
                    m_b = pool.tile([128, cw], bf16, tag="lnmb", name="lnmb", bufs=1)
                    a_b = pool.tile([128, cw], bf16, tag="lnab", name="lnab", bufs=1)
                    nc.gpsimd.partition_broadcast(m_b, m_h)
                    nc.gpsimd.partition_broadcast(a_b, v_h)
                    mb3 = m_b[:, None, :].to_broadcast([128, NDCH, cw])
                    ab3 = a_b[:, None, :].to_broadcast([128, NDCH, cw])
                    nc.vector.tensor_sub(r, r, mb3)
                    if trivial_gb:
                        nc.vector.tensor_mul(o, r, ab3)
                    else:
                        nc.vector.tensor_mul(r, r, ab3)
                        for k in range(NDCH):
                            # host pre-scales lng/lnb by S_K
                            nc.vector.tensor_scalar(o[:, k], r[:, k],
                                                    g_sb[:, k:k + 1], b_sb[:, k:k + 1],
                                                    ALU.mult, ALU.add)

            # ---------- Phase 2: support-side tensors ----------
            ident = small.tile([128, 128], bf16)
            make_identity(nc, ident)
            max_ch = max((int(counts[c]) * T + 127) // 128 for c in range(WAY))
            with tc.tile_pool(name="pp_s", bufs=2, space="PSUM") as pp_s:
                start_item = 0
                for c in range(WAY):
                    n_c = int(counts[c])
                    rows = n_c * T
                    nch = (rows + 127) // 128
                    wlo = offs[c] // 128
                    scratch = sprep.tile([128, 18, max_ch * 128], bf16,
                                         tag="skv", name="skv")
                    if rows < nch * 128:
                        nc.gpsimd.memset(scratch[:, 9:18, rows:nch * 128], 0.0)
                    dst4 = scratch[:, :, :rows].rearrange("p b (n a) -> p b n a", a=T)
                    gather_kv(dst4, start_item, n_c, sprep)
                    # K: LN -> fp8 into s_kT at class cols (pad cols zeroed)
                    pad_lo = offs[c] + rows
                    pad_hi = offs[c + 1] if c + 1 < WAY else nb_pad
                    if pad_hi > pad_lo:
                        nc.gpsimd.memset(s_kT[:, :, pad_lo:pad_hi], 0.0)
                    kraw = scratch[:, 0:9, :rows]
                    col_ln(kraw, rows, s_kT[:, :, offs[c]:offs[c] + rows], sprep, pp_s)
                    # V: transpose class chunks into s_v (fp8, already * S_V)
                    for w in range(nch):
                        for dd in range(NDCH):
                            ps = pp_t.tile([128, 128], bf16, tag="tps")
                            nc.tensor.transpose(
                                ps, scratch[:, 9 + dd, w * 128:(w + 1) * 128], ident)
                            if dd % 2 == 0:
                                nc.vector.tensor_copy(
                                    s_v[:, wlo + w, dd * 128:(dd + 1) * 128], ps)
                            else:
                                nc.scalar.activation(
                                    s_v[:, wlo + w, dd * 128:(dd + 1) * 128],
                                    ps, AF.Copy)
                    start_item += n_c
            pp_t_cm.__exit__(None, None, None)
            sprep_cm.__exit__(None, None, None)

            # ---------- Phase 3: per-group query pipeline ----------
            TS, TB, TC, TA = 0, 1, 2, 3

            def packed_sum(ps_tile, slot, rhs_chunks, first, last, ones_t):
                out = ps_tile[32 * slot:32 * slot + 1, :rhs_chunks[0][0].free_size()]
                for i, (rhs, kc) in enumerate(rhs_chunks):
                    nc.tensor.matmul(out, ones_t[:kc], rhs, start=(first and i == 0),
                                     stop=(last and i == len(rhs_chunks) - 1),
                                     tile_position=(0, 32 * slot),
                                     skip_group_check=True)

            with tc.tile_pool(name="grp", bufs=2) as grp, \
                 tc.tile_pool(name="rows", bufs=2) as rows_pool, \
                 tc.tile_pool(name="pp_sc", bufs=2, space="PSUM") as pp_sc, \
                 tc.tile_pool(name="pp_pr", bufs=2, space="PSUM") as pp_pr, \
                 tc.tile_pool(name="pp_row", bufs=2, space="PSUM") as pp_row:
                items0 = N_SUPPORT
                q_off = 0
                for g, G in enumerate(G_SIZES):
                    C = G * T
                    scr = grp.tile([128, 18, G_MAX, T], bf16, tag="qkv")
                    gather_kv(scr[:, :, :G], items0, G, grp)
                    qk_raw = scr[:, 0:9, :G].rearrange("p m q a -> p m (q a)")
                    qv3 = scr[:, 9:18, :G].rearrange("p m q a -> p m (q a)")
                    qk3 = grp.tile([128, NDCH, C_ALLOC], fp8, tag="qk8")
                    col_ln(qk_raw, C, qk3[:, :, :C], grp, pp_row)

                    # scoresT (fp8: 4 DoubleRow pairs + 1 plain) + exp
                    exp_t = grp.tile([128, nwch, C_ALLOC], fp8, tag="exp")
                    for w in range(nwch):
                        ps = pp_sc.tile([128, C_ALLOC], f32, tag="scps")
                        for k in range(4):
                            nc.tensor.matmul(ps[:, :C],
                                             s_kT[:, 2 * k:2 * k + 2, w * 128:(w + 1) * 128],
                                             qk3[:, 2 * k:2 * k + 2, :C],
                                             start=(k == 0), stop=False, perf_mode=DR)
                        nc.tensor.matmul(ps[:, :C], s_kT[:, 8, w * 128:(w + 1) * 128],
                                         qk3[:, 8, :C], start=False, stop=True)
                        nc.scalar.activation(exp_t[:, w, :C], ps[:, :C], AF.Exp,
                                             scale=inv_sqrt / (S_K * S_K),
                                             bias=expb_sb)

                    packed = rows_pool.tile([WAY, 4, C_ALLOC], f32, tag="packed", bufs=1)

                    # A = ||q_v||^2 per column (qv3 = 4*q_v -> scale 1/16)
                    qsq = grp.tile([128, NDCH, C_ALLOC], bf16, tag="lnsq", bufs=1)
                    nc.vector.tensor_mul(qsq[:, :, :C], qv3, qv3)
                    ps_a = pp_row.tile([128, C_ALLOC], f32, tag="sumps", name="ps_a")
                    packed_sum(ps_a, 0, [(qsq[:, k, :C], 128) for k in range(NDCH)],
                               True, True, ones_bf)
                    a_sb = rows_pool.tile([1, C_ALLOC], f32, tag="a_sb", bufs=1)
                    nc.scalar.activation(a_sb[:, :C], ps_a[0:1, :C], AF.Copy,
                                         scale=1.0 / (S_V * S_V))
                    a5 = rows_pool.tile([WAY, C_ALLOC], f32, tag="a5", bufs=1)
                    nc.gpsimd.partition_broadcast(a5, a_sb)

                    def s_chunks(c):
                        rows = int(counts[c]) * T
                        wlo = offs[c] // 128
                        return [(exp_t[:min(128, rows - wi * 128), wlo + wi, :C],
                                 min(128, rows - wi * 128))
                                for wi in range((rows + 127) // 128)]

                    def stage_rows(ps, rows):
                        st = rows_pool.tile([128, C_ALLOC], f32, tag="stage",
                                            name="stage", bufs=1)
                        nc.scalar.activation(st[:, :C], ps[:, :C], AF.Copy)
                        st4 = st.rearrange("(j z) n -> j z n", z=32)
                        for j, (cc, term) in enumerate(rows):
                            nc.sync.dma_start(packed[cc:cc + 1, term, :C],
                                              st4[j:j + 1, 0, :C])

                    # S_0..S_3 packed in one PSUM tile (concurrent col-groups)
                    ps_s03 = pp_row.tile([128, C_ALLOC], f32, tag="sumps", name="ps_s03")
                    nchunks = max(len(s_chunks(c)) for c in range(4))
                    for i in range(nchunks):
                        for c in range(4):
                            ch = s_chunks(c)
                            if i < len(ch):
                                packed_sum(ps_s03, c, [ch[i]], i == 0, i == len(ch) - 1,
                                           ones_f8)
                    stage_rows(ps_s03, [(0, TS), (1, TS), (2, TS), (3, TS)])

                    ps_bc = pp_row.tile([128, C_ALLOC], f32, tag="sumps", name="ps_bc0")
                    packed_sum(ps_bc, 0, s_chunks(4), True, True, ones_f8)
                    pending = [(4, TS)]

                    def bc_flush(force=False):
                        nonlocal ps_bc, pending
                        if pending and (force or len(pending) >= 3):
                            stage_rows(ps_bc, pending)
                            pending = []
                            if not force:
                                ps_bc = pp_row.tile([128, C_ALLOC], f32, tag="sumps",
                                                    name="ps_bc")
                    for c in range(WAY):
                        rows = int(counts[c]) * T
                        wlo = offs[c] // 128
                        nw_c = (rows + 127) // 128
                        # prototypes: Num[d, col] over class rows (DR pairs + odd single)
                        pt = grp.tile([128, NDCH, C_ALLOC], bf16, tag="pt", bufs=1)
                        p2t = grp.tile([128, NDCH, C_ALLOC], bf16, tag="p2t", bufs=1)
                        for dd in range(NDCH):
                            ps_p = pp_pr.tile([128, C_ALLOC], f32, tag="prps")
                            np_pairs = nw_c // 2
                            for wi in range(np_pairs):
                                nc.tensor.matmul(
                                    ps_p[:, :C],
                                    s_v[:, wlo + 2 * wi:wlo + 2 * wi + 2, dd * 128:(dd + 1) * 128],
                                    exp_t[:, wlo + 2 * wi:wlo + 2 * wi + 2, :C],
                                    start=(wi == 0), stop=(wi == np_pairs - 1 and nw_c % 2 == 0),
                                    perf_mode=DR)
                            if nw_c % 2 == 1:
                                nc.tensor.matmul(
                                    ps_p[:, :C],
                                    s_v[:, wlo + nw_c - 1, dd * 128:(dd + 1) * 128],
                                    exp_t[:, wlo + nw_c - 1, :C],
                                    start=(nw_c == 1), stop=True)
                            # exp_t = exp/8, s_v = 4*v -> ps_p = Num/2
                            if dd % 2 == 0:
                                nc.scalar.activation(pt[:, dd, :C], ps_p[:, :C],
                                                     AF.Copy, scale=2.0)
                            else:
                                nc.vector.tensor_scalar(pt[:, dd, :C], ps_p[:, :C],
                                                        2.0, None, ALU.mult)
                        nc.vector.tensor_mul(p2t[:, :, :C], pt[:, :, :C], pt[:, :, :C])
                        nc.vector.tensor_mul(pt[:, :, :C], pt[:, :, :C], qv3)  # 4*<q_v,Num> terms
                        sb, sc = len(pending), len(pending) + 1
                        for k in range(NDCH):
                            packed_sum(ps_bc, sb, [(pt[:, k, :C], 128)], k == 0,
                                       k == NDCH - 1, ones_bf)
                            packed_sum(ps_bc, sc, [(p2t[:, k, :C], 128)], k == 0,
                                       k == NDCH - 1, ones_bf)
                        pending += [(c, TB), (c, TC)]
                        bc_flush()
                    bc_flush(force=True)

                    # dist = A - B_raw*sinv/16 + C_raw*sinv^2/64 where
                    # sinv = 8/S, B_raw = 4<q_v,Num>, C_raw = ||Num||^2
                    sinv = rows_pool.tile([WAY, C_ALLOC], f32, tag="sinv", bufs=1)
                    nc.vector.reciprocal(sinv[:, :C], packed[:, TS, :C])
                    u = rows_pool.tile([WAY, C_ALLOC], f32, tag="u", bufs=1)
                    nc.vector.tensor_mul(u[:, :C], packed[:, TC, :C], sinv[:, :C])
                    nc.vector.scalar_tensor_tensor(u[:, :C], packed[:, TB, :C], -4.0,
                                                   u[:, :C], ALU.mult, ALU.add)
                    nc.vector.tensor_mul(u[:, :C], u[:, :C], sinv[:, :C])
                    nc.vector.scalar_tensor_tensor(u[:, :C], u[:, :C], 1.0 / 64.0,
                                                   a5[:, :C], ALU.mult, ALU.add)
                    u4 = u[:, :C].rearrange("w (q a) -> w q a", a=T)
                    red = rows_pool.tile([WAY, G_MAX], f32, tag="red", bufs=1)
                    nc.vector.reduce_sum(red[:, :G], u4, mybir.AxisListType.X)
                    nc.scalar.activation(logits5[:, q_off:q_off + G], red[:, :G],
                                         AF.Copy, scale=-1.0 / T)
                    items0 += G
                    q_off += G

            nc.sync.dma_start(out_d.rearrange("q c -> c q"), logits5)

    nc.compile()
    return nc


def kernel(support_set, support_labels, queries, k_w, k_b, v_w, v_b, ln_g, ln_b):
    import concourse.bass_utils as bass_utils

    support_set = np.asarray(support_set, dtype=np.float32)
    queries = np.asarray(queries, dtype=np.float32)
    labels = np.asarray(support_labels, dtype=np.int32)
    k_w = np.asarray(k_w, dtype=np.float32)
    v_w = np.asarray(v_w, dtype=np.float32)
    k_b = np.asarray(k_b, dtype=np.float32)
    v_b = np.asarray(v_b, dtype=np.float32)
    ln_g = np.asarray(ln_g, dtype=np.float32)
    ln_b = np.asarray(ln_b, dtype=np.float32)

    pe = _pos_encoding()
    s = support_set + pe[None]
    q = queries + pe[None]
    order = np.argsort(labels, kind="stable")
    counts = np.bincount(labels, minlength=WAY)
    s_sorted = s[order]
    trivial_gb = bool(np.all(ln_g == 1.0) and np.all(ln_b == 0.0))

    key = (tuple(int(x) for x in counts), trivial_gb)
    if key not in _CACHE:
        _CACHE[key] = _build_kernel(counts, trivial_gb)
    nc = _CACHE[key]

    def to_f8(x):
        return np.clip(x, -240.0, 240.0).astype(F8)

    # weight blocks m = j*18 + kv*9 + dd, scaled by S_W; layout [128, m, kch, 128]
    W = np.zeros((128, NMB, NKCH, 128), np.float32)
    bias = np.zeros((128, NMB), np.float32)
    for kv, (wsrc, bsrc) in enumerate(((k_w, k_b), (v_w, v_b))):
        for j in range(TSS):
            blk = wsrc[j * IN_DIM:(j + 1) * IN_DIM] * S_W   # [2048, 1152]
            blk = blk.reshape(NKCH, 128, NDCH, 128)
            for dd in range(NDCH):
                m = j * 18 + kv * 9 + dd
                W[:, m] = blk[:, :, dd].transpose(1, 0, 2)
                bias[:, m] = bsrc[dd * 128:(dd + 1) * 128] / TSS * (1.0 if kv == 0 else S_V)
    w_perm = to_f8(W)
    g_in = np.ascontiguousarray(ln_g.reshape(NDCH, 128).T * S_K).astype(BF16)
    b_in = np.ascontiguousarray(ln_b.reshape(NDCH, 128).T * S_K).astype(BF16)

    in_maps = []
    for core in range(N_CORES):
        qs = q[core * NQL:(core + 1) * NQL]
        X = np.concatenate([s_sorted.reshape(-1, IN_DIM), qs.reshape(-1, IN_DIM)], 0)
        x_perm = np.ascontiguousarray(
            X.T.reshape(NKCH, 128, NX).transpose(1, 0, 2))
        in_maps.append({"x": to_f8(x_perm), "w": w_perm, "bias": bias,
                        "lng": g_in, "lnb": b_in})

    global _LAST_IN_MAPS
    _LAST_IN_MAPS = in_maps
    res = bass_utils.run_bass_kernel_spmd(nc, in_maps, core_ids=list(range(N_CORES)))
    return np.concatenate([res.results[i]["out"] for i in range(N_CORES)], 0)


_LAST_IN_MAPS = None
